# revision 3
# baseline (speedup 1.0000x reference)
"""BigBird block-sparse attention (3-block sliding window, zero-padded edges)
for Trainium2, SPMD over 8 NeuronCores, data-parallel over the batch dim.

V6: the host pre-transposes x to [B, DIM, N] and pre-casts x and the
weights to bf16 (numerically identical to the in-kernel cast the
baseline did).  The device kernel then needs NO xbar transposes, NO
f32->bf16 casts and NO staging: activations and weights DMA straight
into their compute layouts.  This removes ~60% of all DMA-engine
traffic (the 256B-packet transpose streams) and the cast chains whose
ordering stalled the engine queues.

Per batch element b:
  qT/kT = (x W)^T via W-stationary matmuls over xT, v = x Wv,
  3-block-window attention with exp on ACT and the softmax denominator
  from an appended ones-column of V, output projection with bias.
Matmuls run in bf16 (fp32 accumulation in PSUM).
"""

import os
import numpy as np
import ml_dtypes

import concourse.bass as bass
import concourse.mybir as mybir
import concourse.tile as tile
from concourse import bacc
from concourse.bass_utils import run_bass_kernel_spmd
from concourse.masks import make_identity

B, N, DIM = 16, 1536, 1536
H, DK, DV, BS = 8, 64, 64, 128
NB = N // BS                     # 12 blocks per sequence
NCORES = 8
BPC = B // NCORES                # batch elements per core
SCALE = 1.0 / np.sqrt(DK)        # 0.125

f32 = mybir.dt.float32
bf16 = mybir.dt.bfloat16
EXP = mybir.ActivationFunctionType.Exp

_NC_CACHE = {}
LAST_RESULTS = None


def _emit(nc, reps=1, trace_sim=False,
          psum=(3, 3, 2), evict="dve",
          yb=2, ob=5, y_q="sp", eb=28, rb=4, xgb=4, ot_mode="pe",
          escore=0, epair=False):
    XT = nc.dram_tensor("xT", [BPC, DIM, N], bf16, kind="ExternalInput")
    WQ = nc.dram_tensor("Wq", [DIM, H * DK], bf16, kind="ExternalInput")
    WK = nc.dram_tensor("Wk", [DIM, H * DK], bf16, kind="ExternalInput")
    WV = nc.dram_tensor("Wv", [DIM, H * DV], bf16, kind="ExternalInput")
    WO = nc.dram_tensor("Wo", [H * DV, DIM], bf16, kind="ExternalInput")
    BO = nc.dram_tensor("bo", [DIM], f32, kind="ExternalInput")
    Y = nc.dram_tensor("y", [BPC, N, DIM], f32, kind="ExternalOutput")

    KC = DIM // 128              # 12 contraction chunks for projections
    HV = H * DV                  # 512

    with tile.TileContext(nc, trace_sim=trace_sim) as tc:
        with (
            tc.tile_pool(name="wts", bufs=1) as wts,
            tc.tile_pool(name="xgp", bufs=xgb) as xgp,
            tc.tile_pool(name="qkv", bufs=2) as qkv,
            tc.tile_pool(name="expp", bufs=eb) as expp,
            tc.tile_pool(name="osp", bufs=3) as osp,
            tc.tile_pool(name="otp", bufs=ob) as otp,
            tc.tile_pool(name="rcp", bufs=rb) as rcp,
            tc.tile_pool(name="yp", bufs=yb) as yp,
            tc.tile_pool(name="bigp", bufs=psum[0], space="PSUM") as bigp,
            tc.tile_pool(name="scp", bufs=psum[1], space="PSUM") as scp,
            tc.tile_pool(name="pop", bufs=psum[2], space="PSUM") as pop,
        ):
            # ---- x chunks: one DMA per (elem, 3-block group), straight
            #      into the matmul moving-operand layout ----
            xT_cache = {}

            def load_x(b, i3, nsplit=1):
                r0 = i3 * 3 * BS
                xtr = XT[b].rearrange("(kc p) r -> p kc r", p=128)
                xt = xgp.tile([128, KC, 3 * BS], bf16,
                              name=f"xT{b}_{i3}", tag="xT")
                kstep = KC // nsplit
                for s in range(nsplit):
                    nc.sync.dma_start(
                        out=xt[:, s * kstep:(s + 1) * kstep],
                        in_=xtr[:, s * kstep:(s + 1) * kstep, r0:r0 + 3 * BS])
                xT_cache[(b, i3)] = xt

            load_x(0, 0)

            # ---- weights: direct bf16 DMA per contraction chunk, in
            #      consumption priority order (Wq, Wk, Wv, Wo) ----
            def load_w_chunks(wdram, nm):
                wr = wdram.rearrange("(n p) m -> p n m", p=128)  # [128,12,512]
                chunks = []
                for g in range(KC):
                    ch = wts.tile([128, HV], bf16, name=f"w{nm}{g}")
                    nc.sync.dma_start(out=ch, in_=wr[:, g, :])
                    chunks.append(ch)
                return chunks

            wq_ch = load_w_chunks(WQ, "q")
            load_x(0, 1)
            wk_ch = load_w_chunks(WK, "k")
            wv_ch = load_w_chunks(WV, "v")

            wo_bf = wts.tile([128, HV // 128, DIM], bf16)
            wor = WO.rearrange("(n p) m -> p n m", p=128)          # [128, 4, 1536]
            for c in range(HV // 128):
                nc.sync.dma_start(out=wo_bf[:, c, :], in_=wor[:, c, :])

            # ---- constants ----
            pad128 = wts.tile([128, 1], f32)
            nc.vector.memset(pad128, 128.0)
            ident_bf = wts.tile([128, 128], bf16)
            make_identity(nc, ident_bf)
            bo_bc = wts.tile([128, DIM], f32)
            bo_ap = BO[:]
            nc.sync.dma_start(
                out=bo_bc,
                in_=bass.AP(tensor=bo_ap.tensor, offset=bo_ap.offset,
                            ap=[[0, 128]] + list(bo_ap.ap)),
            )

            def body():
                for b in range(BPC):
                    # per-batch-element activation buffers (transposed layouts)
                    qT_buf = qkv.tile([128, 4, N], bf16, name=f"qT{b}", tag="qT")
                    kT_buf = qkv.tile([128, 4, N], bf16, name=f"kT{b}", tag="kT")
                    v_buf = qkv.tile([128, NB, H, DV + 1], bf16, name=f"v{b}", tag="v")
                    nc.vector.memset(v_buf[:, :, :, DV:DV + 1], 1.0)

                    # expt[(hp, s)][j] = exp tile for head 2hp+s, key block j
                    expt = {(hp, s): [None] * NB
                            for hp in range(H // 2) for s in range(2)}

                    def do_P(i3, b=b):
                        # projections for a triple of row blocks (384 rows)
                        r0 = i3 * 3 * BS
                        if (b, i3) not in xT_cache:
                            load_x(b, i3)
                        xT = xT_cache.pop((b, i3))
                        # prefetch two groups ahead (wraps into next elem)
                        g = b * (NB // 3) + i3 + 2
                        nb_, ni3 = divmod(g, NB // 3)
                        if nb_ < BPC and (nb_, ni3) not in xT_cache:
                            load_x(nb_, ni3)

                        # qT / kT: one psum tile per hv-chunk, N=384 per matmul
                        ev = nc.vector.tensor_copy if evict == "dve" else nc.scalar.copy
                        for (wch, obuf) in ((wq_ch, qT_buf), (wk_ch, kT_buf)):
                            for c in range(4):
                                pp = bigp.tile([128, 3 * BS], f32,
                                               name=f"pp{b}_{i3}_{c}", tag="big")
                                for kc in range(KC):
                                    nc.tensor.matmul(
                                        pp,
                                        wch[kc][:, c * 128:(c + 1) * 128],
                                        xT[:, kc, :],
                                        start=(kc == 0),
                                        stop=(kc == KC - 1),
                                    )
                                ev(obuf[:, c, r0:r0 + 3 * BS], pp)

                        for t in range(3):
                            pv = bigp.tile([128, HV], f32,
                                           name=f"pv{b}_{i3}_{t}", tag="big")
                            for kc in range(KC):
                                nc.tensor.matmul(pv, xT[:, kc, t * BS:(t + 1) * BS],
                                                 wv_ch[kc],
                                                 start=(kc == 0), stop=(kc == KC - 1))
                            ev(v_buf[:, i3 * 3 + t, :, 0:DV],
                               pv.rearrange("p (h d) -> p h d", h=H))

                    def do_scores(hp, s, j, b=b, expt=expt):
                        qlo, qhi = max(j - 1, 0), min(j + 1, NB - 1)
                        nq = (qhi - qlo + 1) * BS
                        pb = s * DK
                        psc = scp.tile([128, 3 * BS], f32,
                                       name=f"psc{b}_{hp}_{j}_{s}", tag="sc")
                        nc.tensor.matmul(
                            psc[:, 0:nq],
                            kT_buf[pb:pb + DK, hp, j * BS:(j + 1) * BS],
                            qT_buf[pb:pb + DK, hp, qlo * BS:(qhi + 1) * BS],
                            start=True, stop=True,
                        )
                        et = expp.tile([128, 3 * BS], bf16,
                                       name=f"et{b}_{hp}_{j}_{s}", tag="exp")
                        nc.scalar.activation(out=et[:, 0:nq], in_=psc[:, 0:nq],
                                             func=EXP, scale=float(SCALE))
                        expt[(hp, s)][j] = et

                    def do_scores_pair(hp, j, b=b, expt=expt):
                        # both 64-row halves into one 2-bank psum tile; one
                        # ACT exp over the pair
                        qlo, qhi = max(j - 1, 0), min(j + 1, NB - 1)
                        nq = (qhi - qlo + 1) * BS
                        psc = scp.tile([128, 2, 512], f32,
                                       name=f"psc{b}_{hp}_{j}", tag="sc")
                        for s in range(2):
                            pb = s * DK
                            nc.tensor.matmul(
                                psc[:, s, 0:nq],
                                kT_buf[pb:pb + DK, hp, j * BS:(j + 1) * BS],
                                qT_buf[pb:pb + DK, hp, qlo * BS:(qhi + 1) * BS],
                                start=True, stop=True,
                            )
                        et = expp.tile([128, 2, 3 * BS], bf16,
                                       name=f"et{b}_{hp}_{j}", tag="exp")
                        nc.scalar.activation(out=et[:, :, 0:nq],
                                             in_=psc[:, :, 0:nq],
                                             func=EXP, scale=float(SCALE))
                        for s in range(2):
                            expt[(hp, s)][j] = et[:, s]

                    def do_attn_out(i, hp, osb_i, b=b, expt=expt):
                        # out[q, dv] for heads 2hp, 2hp+1; sums in col DV
                        po = pop.tile([128, 2, DV + 8], f32,
                                      name=f"po{b}_{hp}_{i}", tag="po")
                        js = [j for j in (i - 1, i, i + 1) if 0 <= j < NB]
                        nmm = len(js) * 2
                        idx = 0
                        for j in js:
                            col = (i - max(j - 1, 0)) * BS
                            for s in range(2):
                                nc.tensor.matmul(
                                    po[:, s, 0:DV + 1],
                                    expt[(hp, s)][j][:, col:col + BS],
                                    v_buf[:, j, hp * 2 + s, :],
                                    start=(idx == 0),
                                    stop=(idx == nmm - 1),
                                )
                                idx += 1
                        if i == 0 or i == NB - 1:
                            # zero-padded edge block: 128 keys with logit 0
                            nc.scalar.activation(
                                out=po[:, 0:2, DV:DV + 1],
                                in_=po[:, 0:2, DV:DV + 1],
                                func=mybir.ActivationFunctionType.Identity,
                                bias=pad128, scale=1.0)
                        rc = rcp.tile([128, 2, 1], f32,
                                      name=f"rc{b}_{hp}_{i}", tag="rc")
                        nc.vector.reciprocal(rc, po[:, 0:2, DV:DV + 1])
                        for s in range(2):
                            nc.vector.tensor_scalar_mul(
                                osb_i[:, (hp * 2 + s) * DV:(hp * 2 + s + 1) * DV],
                                po[:, s, 0:DV], rc[:, s, :])

                    def do_O(i, oT, b=b):
                        # output projection for block i (all heads complete)
                        ysb = yp.tile([128, DIM], f32, name=f"y{b}_{i}", tag="y")
                        for n in range(3):
                            py = bigp.tile([128, 512], f32,
                                           name=f"py{b}_{i}_{n}", tag="big")
                            for c in range(4):
                                nc.tensor.matmul(py, oT[:, c, :],
                                                 wo_bf[:, c, n * 512:(n + 1) * 512],
                                                 start=(c == 0), stop=(c == 3))
                            nc.vector.tensor_add(ysb[:, n * 512:(n + 1) * 512], py,
                                                 bo_bc[:, n * 512:(n + 1) * 512])
                        yeng = nc.sync if y_q == "sp" else nc.scalar
                        yeng.dma_start(out=Y[b, i * BS:(i + 1) * BS, :], in_=ysb)

                    cur = {"j": 0, "i": 0}

                    def advance_scores(hi_j):
                        adv = False
                        while cur["j"] < NB and min(cur["j"] + 1, NB - 1) <= hi_j:
                            for hp in range(H // 2):
                                if epair:
                                    do_scores_pair(hp, cur["j"])
                                else:
                                    for s in range(2):
                                        do_scores(hp, s, cur["j"])
                            cur["j"] += 1
                            adv = True
                        return adv

                    def drain(hi, b=b):
                        while True:
                            acted = advance_scores(hi)
                            while (cur["i"] < NB
                                   and min(cur["i"] + 1, NB - 1) <= cur["j"] - 1):
                                i_cur = cur["i"]
                                osb_i = osp.tile([128, HV], bf16,
                                                 name=f"o{b}_{i_cur}", tag="osb")
                                for hp in range(H // 2):
                                    do_attn_out(i_cur, hp, osb_i)
                                oT = otp.tile([128, 4, BS], bf16,
                                              name=f"oT{b}_{i_cur}", tag="oT")
                                if ot_mode == "dma":
                                    nc.sync.dma_start_transpose(oT, osb_i)
                                else:
                                    for c4 in range(4):
                                        ptp = pop.tile([128, BS], bf16,
                                                       name=f"ptp{b}_{i_cur}_{c4}",
                                                       tag="po")
                                        nc.tensor.transpose(
                                            ptp, osb_i[:, c4 * BS:(c4 + 1) * BS],
                                            ident_bf)
                                        nc.vector.tensor_copy(oT[:, c4, :], ptp)
                                do_O(i_cur, oT)
                                cur["i"] += 1
                                acted = True
                            if not acted:
                                return

                    for i3 in range(NB // 3):
                        do_P(i3)
                        if escore:
                            advance_scores(min(3 * i3 + 1, escore - 1))
                    drain(NB - 1)

            if reps > 1:
                with tc.For_i(0, reps, 1):
                    body()
            else:
                body()


def _get_nc():
    if "nc" not in _NC_CACHE:
        nc = bacc.Bacc("TRN2", target_bir_lowering=False, debug=False)
        _emit(nc)
        nc.finalize()
        _NC_CACHE["nc"] = nc
    return _NC_CACHE["nc"]


def _prep(x, Wq, Wk, Wv, Wo, bo):
    bf = ml_dtypes.bfloat16
    xT = np.ascontiguousarray(
        np.transpose(np.asarray(x, np.float32), (0, 2, 1))).astype(bf)
    return (xT,
            np.ascontiguousarray(np.asarray(Wq, np.float32)).astype(bf),
            np.ascontiguousarray(np.asarray(Wk, np.float32)).astype(bf),
            np.ascontiguousarray(np.asarray(Wv, np.float32)).astype(bf),
            np.ascontiguousarray(np.asarray(Wo, np.float32)).astype(bf),
            np.ascontiguousarray(np.asarray(bo, np.float32)))


def kernel(x, Wq, Wk, Wv, Wo, bo):
    global LAST_RESULTS
    xT, Wqb, Wkb, Wvb, Wob, bo = _prep(x, Wq, Wk, Wv, Wo, bo)

    nc = _get_nc()
    in_maps = [
        {"xT": xT[c * BPC:(c + 1) * BPC], "Wq": Wqb, "Wk": Wkb, "Wv": Wvb,
         "Wo": Wob, "bo": bo}
        for c in range(NCORES)
    ]
    trace = bool(int(os.environ.get("KERNEL_TRACE", "0")))
    res = run_bass_kernel_spmd(nc, in_maps, list(range(NCORES)), trace=trace)
    LAST_RESULTS = res
    return np.concatenate([res.results[c]["y"] for c in range(NCORES)], axis=0)
